# revision 19
# baseline (speedup 1.0000x reference)
"""Trainium2 Bass kernel for DenseCRFLoss.

Computes  loss = WEIGHT * (-1/B) * sum_b  sum_{k,i,j} S[b,k,i] K_b[i,j] S[b,k,j]
where K_b[i,j] = exp(-0.5*||f_i - f_j||^2) is the joint bilateral kernel over
downsampled positions+colors (P = 96*96 = 9216 pixels per image).

Device strategy (v3, the default; v2/v1 kept for reference):
  * Features are augmented so that  fhat_i . ghat_j = -0.5*d2(i,j) in ONE
    matmul (MM1). For full precision the fp32 features are hi/lo-split into
    bf16 pairs (21-row contraction = hi.hi + hi.lo + lo.hi); the +-900
    magnitude terms cancel in PSUM fp32.
  * Symmetry: loss = 2*sum_{i<j} + sum_diag. Block-column J of an image
    gives J+1 uniform segments of 4 [128x512] tiles (w=2 above the
    diagonal, w=1 for the diagonal band); weights fold into the SJ stream.
    684 real segments + 4 dummies spread over 8 cores (NSEG=86 slots each).
  * exp is the throughput floor (one entry/cycle/lane on ACT only), so it
    is SPLIT between the scalar engine (true Exp -> bf16) and the vector
    engine: DVE groups use a Schraudolph-style fast exp -- a single
    tensor_scalar(mult,add) with fp32->uint16 saturating convert whose bit
    pattern IS the bf16 kernel value (t = x*128*log2e + 16249; negative t
    saturates to 0 = correct far-pair kernel; HW convert verified
    round-to-nearest, matching the numpy-tuned constant: +3e-4 bias at
    100% share). Share ~5/12 of groups balances ACT vs DVE busy.
  * MM2 is FLIPPED: kt tile [128,128] is the stationary (128 weight cols
    -> FWL-eligible) and S^T [128,21] is the moving operand, so MM2 costs
    only 4x21 moving columns per tile (6x less PE time than the natural
    orientation) and the As accumulator becomes [128, 4*21] in ONE PSUM
    bank. PSUM start=True zeroes the whole 2KB bank (HW-verified), so only
    the first matmul per segment starts; the other 15 accumulate.
  * Final reduce: one DVE scalar_tensor_tensor per segment (As * SJ with
    accum_out = row-sum along free dim) -> accv column; host sums.
  * 2-tile exp groups, 3 dot PSUM bufs, 4-group software pipeline so PE
    never head-of-line blocks behind exp; MM2s trail their group by 4.
  * Cost model 130.4us, HW slope ~122us (v2 was 203.6us / ~176us): ~1.45x.
"""

import numpy as np
import ml_dtypes
from contextlib import ExitStack

import concourse.bass as bass
from concourse import bacc
import concourse.tile as tile
from concourse.mybir import dt, ActivationFunctionType, AluOpType, AxisListType
from concourse.bass_utils import run_bass_kernel_spmd

# ---- problem constants (hardcoded; kernel.py must be self-contained) ----
B = 4
KCH = 21
HH = 96                   # downsampled H=W
P = HH * HH               # 9216 pixels
NCORES = 8
HALF = P // 2             # 4608 columns per core
NI = P // 128             # 72 i-chunks
NJ = HALF // 512          # 9 j-chunks per core
GRP = 3                   # i-chunks per exp group (3 PSUM banks)
SIGMA_RGB = 15.0
SXY_EFF = 100.0 * 0.5     # sigma_xy * scale_factor
WEIGHT = 2e-9

MM1_MODE = "bf16split"         # "f32r" | "bf16split"

_cache = {}


def _build_nc(mm1_mode, reps=1):
    nc = bacc.Bacc("TRN2", target_bir_lowering=False)
    nf = 7 if mm1_mode == "f32r" else 21
    mm1_dt = dt.float32r if mm1_mode == "f32r" else dt.bfloat16

    fT = nc.dram_tensor("fT", [nf, P], mm1_dt, kind="ExternalInput")
    gT = nc.dram_tensor("gT", [nf, HALF], mm1_dt, kind="ExternalInput")
    sT = nc.dram_tensor("sT", [128, NI * KCH], dt.bfloat16, kind="ExternalInput")
    sj = nc.dram_tensor("sj", [KCH, HALF], dt.float32, kind="ExternalInput")
    out = nc.dram_tensor("out", [KCH, NJ], dt.float32, kind="ExternalOutput")

    with tile.TileContext(nc) as tc, ExitStack() as ctx:
        cpool = ctx.enter_context(tc.tile_pool(name="const", bufs=1))
        f_sb = cpool.tile([nf, P], mm1_dt)
        nc.gpsimd.dma_start(f_sb[:], fT[:])
        g_sb = cpool.tile([nf, HALF], mm1_dt)
        nc.gpsimd.dma_start(g_sb[:], gT[:])
        sT_sb = cpool.tile([128, NI * KCH], dt.bfloat16)
        nc.gpsimd.dma_start(sT_sb[:], sT[:])
        sj_sb = cpool.tile([KCH, HALF], dt.float32)
        nc.gpsimd.dma_start(sj_sb[:], sj[:])
        accv = cpool.tile([KCH, NJ], dt.float32)

        dpool = ctx.enter_context(tc.tile_pool(name="dot", bufs=2, space="PSUM"))
        apool = ctx.enter_context(tc.tile_pool(name="asum", bufs=2, space="PSUM"))
        kpool = ctx.enter_context(tc.tile_pool(name="ktile", bufs=3))
        spool = ctx.enter_context(tc.tile_pool(name="scr", bufs=2))

        for rep in range(reps):
            for jb in range(NJ):
                As = apool.tile([KCH, 512], dt.float32, tag="As")
                g_slice = g_sb[:, jb * 512:(jb + 1) * 512]
                for g in range(NI // GRP):
                    dot = dpool.tile([128, GRP * 512], dt.float32, tag="dot")
                    for t in range(GRP):
                        ib = g * GRP + t
                        nc.tensor.matmul(
                            dot[:, t * 512:(t + 1) * 512],
                            f_sb[:, ib * 128:(ib + 1) * 128],
                            g_slice,
                            start=True, stop=True,
                        )
                    kt = kpool.tile([128, GRP * 512], dt.bfloat16, tag="kt")
                    nc.scalar.activation(kt[:], dot[:], ActivationFunctionType.Exp)
                    for t in range(GRP):
                        ib = g * GRP + t
                        nc.tensor.matmul(
                            As[:],
                            sT_sb[:, ib * KCH:(ib + 1) * KCH],
                            kt[:, t * 512:(t + 1) * 512],
                            start=(ib == 0), stop=(ib == NI - 1),
                        )
                scr = spool.tile([KCH, 512], dt.float32, tag="scr")
                nc.vector.tensor_mul(scr[:], As[:],
                                     sj_sb[:, jb * 512:(jb + 1) * 512])
                nc.vector.reduce_sum(accv[:, jb:jb + 1], scr[:],
                                     axis=AxisListType.X)
        nc.sync.dma_start(out[:], accv[:])
    nc.finalize()
    return nc


def _split_bf16(x):
    hi = x.astype(ml_dtypes.bfloat16)
    lo = (x - hi.astype(np.float32)).astype(ml_dtypes.bfloat16)
    return hi, lo


def _prep_inputs(segmentations, images, mm1_mode):
    seg = np.asarray(segmentations, dtype=np.float32)
    img = np.asarray(images, dtype=np.float32)
    S = seg.reshape(B, KCH, HH, 2, HH, 2).mean(axis=(3, 5)).reshape(B, KCH, P)
    rgb = img[:, :, ::2, ::2].reshape(B, 3, P)

    yy, xx = np.meshgrid(np.arange(HH, dtype=np.float32),
                         np.arange(HH, dtype=np.float32), indexing="ij")
    pos = np.stack([xx.ravel(), yy.ravel()], axis=0) / SXY_EFF  # [2, P]

    in_maps = []
    for b in range(B):
        feat = np.concatenate([pos, rgb[b] / SIGMA_RGB], axis=0).astype(np.float32)
        msq = -0.5 * (feat * feat).sum(axis=0, dtype=np.float32)   # [P]
        ones = np.ones((1, P), np.float32)
        fhat = np.concatenate([feat, ones, msq[None, :]], axis=0)  # [7, P]
        ghat = np.concatenate([feat, msq[None, :], ones], axis=0)  # [7, P]
        if mm1_mode == "f32r":
            fT_full = fhat
            gT_full = ghat
        else:
            fhi, flo = _split_bf16(fhat)
            ghi, glo = _split_bf16(ghat)
            fT_full = np.concatenate([fhi, fhi, flo], axis=0)      # [21, P]
            gT_full = np.concatenate([ghi, glo, ghi], axis=0)      # [21, P]
        sT = np.ascontiguousarray(
            S[b].reshape(KCH, NI, 128).transpose(2, 1, 0).reshape(128, NI * KCH)
        ).astype(ml_dtypes.bfloat16)
        for h in range(2):
            sl = slice(h * HALF, (h + 1) * HALF)
            in_maps.append({
                "fT": np.ascontiguousarray(fT_full),
                "gT": np.ascontiguousarray(gT_full[:, sl]),
                "sT": sT,
                "sj": np.ascontiguousarray(S[b][:, sl]),
            })
    return in_maps


# ---------------- v2: symmetric (upper-triangle) kernel ----------------
# Per image, loss = 2*sum_{i<j} q_ij + sum_diag. Block-columns J (512 wide,
# 18 per image) decompose into J+1 uniform segments of 4 tiles (128 rows
# each): k<J strictly above the diagonal (weight 2), k==J the diagonal band
# (weight 1). Weights are folded into the pre-multiplied SJ stream, so the
# SPMD program is a flat, fully data-driven list of NSEG uniform segments.
NSEG = 90          # segment slots per core (684 real across 8 cores + dummies)
TPS = 4            # tiles per segment
CST = 6            # segments staged per DMA chunk
NTILE = NSEG * TPS # 360
NGRP = NTILE // GRP  # 120 exp groups of 3 tiles

def _build_nc_v2(reps=1):
    nc = bacc.Bacc("TRN2", target_bir_lowering=False)
    fD = nc.dram_tensor("fD", [21, NTILE * 128], dt.bfloat16, kind="ExternalInput")
    gD = nc.dram_tensor("gD", [21, NSEG * 512], dt.bfloat16, kind="ExternalInput")
    sjD = nc.dram_tensor("sjD", [KCH, NSEG * 512], dt.float32, kind="ExternalInput")
    stD = nc.dram_tensor("stD", [128, NTILE * KCH], dt.bfloat16, kind="ExternalInput")
    out = nc.dram_tensor("out", [KCH, NSEG], dt.float32, kind="ExternalOutput")

    with tile.TileContext(nc) as tc, ExitStack() as ctx:
        cpool = ctx.enter_context(tc.tile_pool(name="const", bufs=1))
        f_sb = cpool.tile([21, NTILE * 128], dt.bfloat16)
        nc.gpsimd.dma_start(f_sb[:], fD[:])
        st_sb = cpool.tile([128, NTILE * KCH], dt.bfloat16)
        nc.gpsimd.dma_start(st_sb[:], stD[:])
        accv = cpool.tile([KCH, NSEG], dt.float32)

        gpool = ctx.enter_context(tc.tile_pool(name="gstage", bufs=3))
        sjpool = ctx.enter_context(tc.tile_pool(name="sjstage", bufs=3))
        dpool = ctx.enter_context(tc.tile_pool(name="dot", bufs=2, space="PSUM"))
        apool = ctx.enter_context(tc.tile_pool(name="asum", bufs=2, space="PSUM"))
        kpool = ctx.enter_context(tc.tile_pool(name="ktile", bufs=3))
        spool = ctx.enter_context(tc.tile_pool(name="scr", bufs=2))

        for rep in range(reps):
            gst = {}
            sjst = {}
            As = None
            for g in range(NGRP):
                dot = dpool.tile([128, GRP * 512], dt.float32, tag="dot")
                for t in range(GRP):
                    T = g * GRP + t
                    s, pos = divmod(T, TPS)
                    c, sloc = divmod(s, CST)
                    if pos == 0 and sloc == 0:
                        gst[c] = gpool.tile([21, CST * 512], dt.bfloat16,
                                            tag="gs", name="gs")
                        nc.sync.dma_start(
                            gst[c][:],
                            gD[:, c * CST * 512:(c + 1) * CST * 512])
                        sjst[c] = sjpool.tile([KCH, CST * 512], dt.float32,
                                              tag="sjs", name="sjs")
                        nc.sync.dma_start(
                            sjst[c][:],
                            sjD[:, c * CST * 512:(c + 1) * CST * 512])
                    nc.tensor.matmul(
                        dot[:, t * 512:(t + 1) * 512],
                        f_sb[:, T * 128:(T + 1) * 128],
                        gst[c][:, sloc * 512:(sloc + 1) * 512],
                        start=True, stop=True,
                    )
                kt = kpool.tile([128, GRP * 512], dt.bfloat16, tag="kt")
                nc.scalar.activation(kt[:], dot[:], ActivationFunctionType.Exp)
                for t in range(GRP):
                    T = g * GRP + t
                    s, pos = divmod(T, TPS)
                    if pos == 0:
                        As = apool.tile([KCH, 512], dt.float32, tag="As")
                    nc.tensor.matmul(
                        As[:],
                        st_sb[:, T * KCH:(T + 1) * KCH],
                        kt[:, t * 512:(t + 1) * 512],
                        start=(pos == 0), stop=(pos == TPS - 1),
                    )
                    if pos == TPS - 1:
                        c, sloc = divmod(s, CST)
                        scr = spool.tile([KCH, 512], dt.float32, tag="scr")
                        nc.vector.tensor_mul(
                            scr[:], As[:],
                            sjst[c][:, sloc * 512:(sloc + 1) * 512])
                        nc.vector.reduce_sum(accv[:, s:s + 1], scr[:],
                                             axis=AxisListType.X)
        nc.sync.dma_start(out[:], accv[:])
    nc.finalize()
    return nc


def _prep_inputs_v2(segmentations, images):
    seg = np.asarray(segmentations, dtype=np.float32)
    img = np.asarray(images, dtype=np.float32)
    S = seg.reshape(B, KCH, HH, 2, HH, 2).mean(axis=(3, 5)).reshape(B, KCH, P)
    rgb = img[:, :, ::2, ::2].reshape(B, 3, P)
    yy, xx = np.meshgrid(np.arange(HH, dtype=np.float32),
                         np.arange(HH, dtype=np.float32), indexing="ij")
    pos = np.stack([xx.ravel(), yy.ravel()], axis=0) / SXY_EFF

    seglist = [(J, k, 2.0 if k < J else 1.0)
               for J in range(18) for k in range(J + 1)]   # 171 per image
    dummy = (0, 0, 0.0)

    in_maps = []
    for b in range(B):
        feat = np.concatenate([pos, rgb[b] / SIGMA_RGB], axis=0).astype(np.float32)
        msq = -0.5 * (feat * feat).sum(axis=0, dtype=np.float32)
        ones = np.ones((1, P), np.float32)
        fhat = np.concatenate([feat, ones, msq[None, :]], axis=0)
        ghat = np.concatenate([feat, msq[None, :], ones], axis=0)
        fhi, flo = _split_bf16(fhat)
        ghi, glo = _split_bf16(ghat)
        fT_full = np.concatenate([fhi, fhi, flo], axis=0)   # [21, P] bf16
        gT_full = np.concatenate([ghi, glo, ghi], axis=0)   # [21, P] bf16
        sT_all = np.ascontiguousarray(
            S[b].reshape(KCH, NI, 128).transpose(2, 1, 0).reshape(128, NI * KCH)
        ).astype(ml_dtypes.bfloat16)
        for h in range(2):
            segs = (seglist[:NSEG] + [dummy] * (NSEG - min(NSEG, 171))
                    if h == 0 else
                    seglist[NSEG:] + [dummy] * (2 * NSEG - 171))
            assert len(segs) == NSEG
            fDl, gDl, sjDl, stDl = [], [], [], []
            for (J, k, w) in segs:
                fDl.append(fT_full[:, 512 * k:512 * (k + 1)])
                gDl.append(gT_full[:, 512 * J:512 * (J + 1)])
                sjDl.append(np.float32(w) * S[b][:, 512 * J:512 * (J + 1)])
                stDl.append(sT_all[:, 84 * k:84 * (k + 1)])
            in_maps.append({
                "fD": np.ascontiguousarray(np.concatenate(fDl, axis=1)),
                "gD": np.ascontiguousarray(np.concatenate(gDl, axis=1)),
                "sjD": np.ascontiguousarray(
                    np.concatenate(sjDl, axis=1).astype(np.float32)),
                "stD": np.ascontiguousarray(np.concatenate(stDl, axis=1)),
            })
    return in_maps


# ---------------- v3: ACT/DVE exp split + [117,128] As layout ----------------
# Changes over v2:
#   * NSEG 90 -> 86 (684 real segments + 4 dummies over 8 cores).
#   * exp split between the scalar engine (ACT, bf16 out) and the vector
#     engine (DVE): DVE groups use a Schraudolph-style fast exp -- one
#     tensor_scalar(mult,add) with fp32->uint16 saturating convert whose
#     result bit-pattern IS the bf16 kernel value (t = x*128*log2e +
#     (127*128 + C_SCHRAUD)); negative t saturates to 0, which is the
#     correct kernel value for pairs that far apart. Constant tuned on the
#     real input distribution (rel bias +3e-4 at 100% DVE share).
#   * As accumulator [21,512] -> [117,128]: MM2 becomes 4 column-chunk
#     matmuls at tile_position column offsets 0/32/64/96, so the SJ
#     multiply-reduce runs over free-dim 128 instead of 512 and fuses into
#     ONE DVE scalar_tensor_tensor with accum_out (sum along free dim).
NSEG3 = 86
NTILE3 = NSEG3 * TPS          # 344
NGRP3 = (NTILE3 + GRP - 1) // GRP   # 115 (last group has 2 tiles)
DVE_NUM, DVE_DEN = 5, 12      # ~0.42 of exp groups routed to DVE
A_SCHRAUD = 184.66299622      # 128/ln(2)
PBASE = [0, 32, 64, 0]        # As chunk partition bases (HW allows 0/32/64)
CBASE = [0, 0, 0, 128]        # As chunk column bases
MM2_FLIP = True               # kt stationary / st moving: As' = [128j, 4*21]
B_SCHRAUD = 16249.0           # 127*128 - 7.0 (tuned on the input dist)


def _route_dve(g):
    return (g * DVE_NUM) % DVE_DEN < DVE_NUM


def _build_nc_v3(reps=1, grp=2, dot_bufs=3, kt_bufs=7, delay=4,
                 as_bufs=2, mm2_flip=True):
    grp = GRP if grp is None else grp
    nc = bacc.Bacc("TRN2", target_bir_lowering=False)
    fD = nc.dram_tensor("fD", [21, NTILE3 * 128], dt.bfloat16, kind="ExternalInput")
    gD = nc.dram_tensor("gD", [21, NSEG3 * 512], dt.bfloat16, kind="ExternalInput")
    sjw = 84 if MM2_FLIP else 256
    sjp = 128 if MM2_FLIP else 85
    sjD = nc.dram_tensor("sjD", [sjp, NSEG3 * sjw], dt.bfloat16, kind="ExternalInput")
    stD = nc.dram_tensor("stD", [128, NTILE3 * KCH], dt.bfloat16, kind="ExternalInput")
    out = nc.dram_tensor("out", [sjp, NSEG3], dt.float32, kind="ExternalOutput")

    with tile.TileContext(nc) as tc, ExitStack() as ctx:
        cpool = ctx.enter_context(tc.tile_pool(name="const", bufs=1))
        f_sb = cpool.tile([21, NTILE3 * 128], dt.bfloat16)
        for fq in range(4):   # split so early MM1s unblock before full load
            fsl = slice(fq * NTILE3 * 32, (fq + 1) * NTILE3 * 32)
            nc.gpsimd.dma_start(f_sb[:, fsl], fD[:, fsl])
        st_sb = cpool.tile([128, NTILE3 * KCH], dt.bfloat16)
        nc.gpsimd.dma_start(st_sb[:], stD[:])
        accv = cpool.tile([sjp, NSEG3], dt.float32)
        nc.vector.memset(accv[:], 0.0)

        gpool = ctx.enter_context(tc.tile_pool(name="gstage", bufs=3))
        sjpool = ctx.enter_context(tc.tile_pool(name="sjstage", bufs=3))
        dpool = ctx.enter_context(tc.tile_pool(name="dot", bufs=dot_bufs, space="PSUM"))
        apool = ctx.enter_context(tc.tile_pool(name="asum", bufs=as_bufs, space="PSUM"))
        kpool = ctx.enter_context(tc.tile_pool(name="ktile", bufs=kt_bufs))
        spool = ctx.enter_context(tc.tile_pool(name="scr", bufs=2))

        for rep in range(reps):
            gst = {}
            sjst = {}
            state = {"As": None, "kt": {}}

            def stage_mm1_exp(g):
                ntg = min(grp, NTILE3 - g * grp)
                dot = dpool.tile([128, grp * 512], dt.float32, tag="dot",
                                 name="dot")
                for t in range(ntg):
                    T = g * grp + t
                    s, pos = divmod(T, TPS)
                    c, sloc = divmod(s, CST)
                    if pos == 0 and sloc == 0:
                        nsc = min(CST, NSEG3 - c * CST)
                        gst[c] = gpool.tile([21, CST * 512], dt.bfloat16,
                                            tag="gs", name="gs")
                        nc.sync.dma_start(
                            gst[c][:, :nsc * 512],
                            gD[:, c * CST * 512:c * CST * 512 + nsc * 512])
                        sjst[c] = sjpool.tile([sjp, CST * sjw], dt.bfloat16,
                                              tag="sjs", name="sjs")
                        nc.sync.dma_start(
                            sjst[c][:, :nsc * sjw],
                            sjD[:, c * CST * sjw:c * CST * sjw + nsc * sjw])
                    nc.tensor.matmul(
                        dot[:, t * 512:(t + 1) * 512],
                        f_sb[:, T * 128:(T + 1) * 128],
                        gst[c][:, sloc * 512:(sloc + 1) * 512],
                        start=True, stop=True,
                    )
                kt = kpool.tile([128, grp * 512], dt.bfloat16, tag="kt",
                                name="kt")
                if _route_dve(g):
                    nc.vector.tensor_scalar(
                        kt[:, :ntg * 512].bitcast(dt.uint16),
                        dot[:, :ntg * 512],
                        A_SCHRAUD, B_SCHRAUD,
                        op0=AluOpType.mult, op1=AluOpType.add,
                    )
                else:
                    nc.scalar.activation(kt[:, :ntg * 512], dot[:, :ntg * 512],
                                         ActivationFunctionType.Exp)
                state["kt"][g] = kt

            def stage_mm2(g):
                ntg = min(grp, NTILE3 - g * grp)
                kt = state["kt"].pop(g)
                for t in range(ntg):
                    T = g * grp + t
                    s, pos = divmod(T, TPS)
                    if pos == 0:
                        # full-bank tile: PSUM start=True zeroes the whole
                        # 2KB bank, so (a) each buf must own its bank and
                        # (b) only the FIRST matmul per segment may start
                        state["As"] = apool.tile(
                            [128, 512], dt.float32, tag="As", name="As")
                    As = state["As"]
                    for q in range(4):
                        if MM2_FLIP:
                            # kt chunk stationary (128 weight cols -> FWL),
                            # st moving (21 cols): As'[j, 21q+k]
                            nc.tensor.matmul(
                                As[:, q * KCH:(q + 1) * KCH],
                                kt[:, t * 512 + q * 128:t * 512 + (q + 1) * 128],
                                st_sb[:, T * KCH:(T + 1) * KCH],
                                start=(pos == 0 and q == 0),
                                stop=(pos == TPS - 1),
                                skip_group_check=True,
                            )
                        else:
                            pb, cb = PBASE[q], CBASE[q]
                            nc.tensor.matmul(
                                As[pb:pb + KCH, cb:cb + 128],
                                st_sb[:, T * KCH:(T + 1) * KCH],
                                kt[:, t * 512 + q * 128:t * 512 + (q + 1) * 128],
                                start=(pos == 0), stop=(pos == TPS - 1),
                            )
                    if pos == TPS - 1:
                        c, sloc = divmod(s, CST)
                        scr = spool.tile([sjp, sjw], dt.bfloat16, tag="scr",
                                         name="scr")
                        nc.vector.scalar_tensor_tensor(
                            scr[:], As[:, :sjw], 1.0,
                            sjst[c][:, sloc * sjw:(sloc + 1) * sjw],
                            op0=AluOpType.mult, op1=AluOpType.mult,
                            accum_out=accv[:, s:s + 1],
                        )

            # software pipeline, `delay` groups deep; MM2s emitted
            # before MM1s so accums aren't queued behind the next exp
            ngrp = (NTILE3 + grp - 1) // grp
            for g in range(delay):
                stage_mm1_exp(g)
            for g in range(delay, ngrp):
                stage_mm2(g - delay)
                stage_mm1_exp(g)
            for g in range(ngrp - delay, ngrp):
                stage_mm2(g)
        nc.sync.dma_start(out[:], accv[:])
    nc.finalize()
    return nc


def _prep_inputs_v3(segmentations, images):
    seg = np.asarray(segmentations, dtype=np.float32)
    img = np.asarray(images, dtype=np.float32)
    S = seg.reshape(B, KCH, HH, 2, HH, 2).mean(axis=(3, 5)).reshape(B, KCH, P)
    rgb = img[:, :, ::2, ::2].reshape(B, 3, P)
    yy, xx = np.meshgrid(np.arange(HH, dtype=np.float32),
                         np.arange(HH, dtype=np.float32), indexing="ij")
    pos = np.stack([xx.ravel(), yy.ravel()], axis=0) / SXY_EFF

    fTs, gTs, sTs = [], [], []
    for b in range(B):
        feat = np.concatenate([pos, rgb[b] / SIGMA_RGB], axis=0).astype(np.float32)
        msq = -0.5 * (feat * feat).sum(axis=0, dtype=np.float32)
        ones = np.ones((1, P), np.float32)
        fhat = np.concatenate([feat, ones, msq[None, :]], axis=0)
        ghat = np.concatenate([feat, msq[None, :], ones], axis=0)
        fhi, flo = _split_bf16(fhat)
        ghi, glo = _split_bf16(ghat)
        fTs.append(np.concatenate([fhi, fhi, flo], axis=0))   # [21, P] bf16
        gTs.append(np.concatenate([ghi, glo, ghi], axis=0))   # [21, P] bf16
        sTs.append(np.ascontiguousarray(
            S[b].reshape(KCH, NI, 128).transpose(2, 1, 0).reshape(128, NI * KCH)
        ).astype(ml_dtypes.bfloat16))

    # global flat segment list: (b, J, k, w); 684 real + 4 dummies = 8*86
    seglist = [(b, J, k, 2.0 if k < J else 1.0)
               for b in range(B) for J in range(18) for k in range(J + 1)]
    seglist += [(0, 0, 0, 0.0)] * (NCORES * NSEG3 - len(seglist))

    in_maps = []
    for core in range(NCORES):
        segs_c = seglist[core * NSEG3:(core + 1) * NSEG3]
        fDl, gDl, sjDl, stDl = [], [], [], []
        for (b, J, k, w) in segs_c:
            fDl.append(fTs[b][:, 512 * k:512 * (k + 1)])
            gDl.append(gTs[b][:, 512 * J:512 * (J + 1)])
            if MM2_FLIP:
                # As'[j, 21q+k] -> sjb[j, 21q+k] = w*S[k, 512J+128q+j]
                blk = S[b][:, 512 * J:512 * (J + 1)]        # [21, 512]
                sjb = np.float32(w) * np.ascontiguousarray(
                    blk.reshape(KCH, 4, 128).transpose(2, 1, 0).reshape(128, 84))
            else:
                sjb = np.zeros((85, 256), np.float32)
                for q in range(4):
                    sjb[PBASE[q]:PBASE[q] + KCH, CBASE[q]:CBASE[q] + 128] = \
                        np.float32(w) * S[b][:, 512 * J + 128 * q:512 * J + 128 * (q + 1)]
            sjDl.append(sjb)
            stDl.append(sTs[b][:, 84 * k:84 * (k + 1)])
        in_maps.append({
            "fD": np.ascontiguousarray(np.concatenate(fDl, axis=1)),
            "gD": np.ascontiguousarray(np.concatenate(gDl, axis=1)),
            "sjD": np.ascontiguousarray(
                np.concatenate(sjDl, axis=1).astype(ml_dtypes.bfloat16)),
            "stD": np.ascontiguousarray(np.concatenate(stDl, axis=1)),
        })
    return in_maps


KERNEL_V = 3


def kernel(segmentations, images, _trace=False):
    if KERNEL_V == 3:
        key = "v3"
        if key not in _cache:
            _cache[key] = _build_nc_v3()
        nc = _cache[key]
        in_maps = _prep_inputs_v3(segmentations, images)
    elif KERNEL_V == 2:
        key = "v2"
        if key not in _cache:
            _cache[key] = _build_nc_v2()
        nc = _cache[key]
        in_maps = _prep_inputs_v2(segmentations, images)
    else:
        key = MM1_MODE
        if key not in _cache:
            _cache[key] = _build_nc(MM1_MODE)
        nc = _cache[key]
        in_maps = _prep_inputs(segmentations, images, MM1_MODE)
    res = run_bass_kernel_spmd(nc, in_maps, core_ids=list(range(NCORES)),
                               trace=_trace)
    kernel._last_results = res
    if KERNEL_V == 3:
        if MM2_FLIP:
            total = sum(float(np.asarray(r["out"], dtype=np.float64).sum())
                        for r in res.results)
        else:
            rows = np.r_[0:KCH, 32:32 + KCH, 64:64 + KCH]
            total = sum(float(np.asarray(r["out"], dtype=np.float64)[rows].sum())
                        for r in res.results)
    else:
        total = sum(float(np.asarray(r["out"], dtype=np.float64).sum())
                    for r in res.results)
    return np.asarray(np.float32(-WEIGHT * total / B))


def _make_timer(nc, in_maps, timing_reps):
    """Build the jitted SPMD executor for `nc` (mirrors
    bass2jax.run_bass_via_pjrt multi-core path) with device-resident inputs;
    return min wall-clock ns over `timing_reps` calls."""
    import time
    import jax
    from jax.sharding import Mesh, PartitionSpec, NamedSharding
    from jax.experimental.shard_map import shard_map
    import concourse.mybir as mybir
    from concourse import bass2jax

    bass2jax.install_neuronx_cc_hook()
    partition_name = nc.partition_id_tensor.name if nc.partition_id_tensor else None
    in_names, out_names, out_avals, zero_outs = [], [], [], []
    for alloc in nc.m.functions[0].allocations:
        if not isinstance(alloc, mybir.MemoryLocationSet):
            continue
        name = alloc.memorylocations[0].name
        if alloc.kind == "ExternalInput":
            if name != partition_name:
                in_names.append(name)
        elif alloc.kind == "ExternalOutput":
            out_names.append(name)
            shape = tuple(alloc.tensor_shape)
            dtype = mybir.dt.np(alloc.dtype)
            out_avals.append(jax.core.ShapedArray(shape, dtype))
            zero_outs.append(np.zeros(shape, dtype))
    n_params = len(in_names)

    def _body(*args):
        operands = list(args)
        if partition_name is not None:
            operands.append(bass2jax.partition_id_tensor())
        outs = bass2jax._bass_exec_p.bind(
            *operands,
            out_avals=tuple(out_avals),
            in_names=tuple(in_names + out_names
                           + ([partition_name] if partition_name else [])),
            out_names=tuple(out_names),
            lowering_input_output_aliases=(),
            sim_require_finite=True,
            sim_require_nnan=True,
            nc=nc,
        )
        return tuple(outs)

    devices = jax.devices()[:NCORES]
    mesh = Mesh(np.asarray(devices), ("core",))
    in_specs = (PartitionSpec("core"),) * (n_params + len(out_names))
    out_specs = (PartitionSpec("core"),) * len(out_names)
    sharded = jax.jit(
        shard_map(_body, mesh=mesh, in_specs=in_specs, out_specs=out_specs,
                  check_rep=False),
        keep_unused=True,
    )
    per_core = [[np.asarray(m[name]) for name in in_names] for m in in_maps]
    concat_in = [
        jax.device_put(
            np.concatenate([per_core[c][i] for c in range(NCORES)], axis=0),
            NamedSharding(mesh, PartitionSpec("core")))
        for i in range(n_params)
    ]
    concat_zeros = [
        jax.device_put(np.zeros((NCORES * z.shape[0], *z.shape[1:]), z.dtype),
                       NamedSharding(mesh, PartitionSpec("core")))
        for z in zero_outs
    ]
    out = sharded(*concat_in, *concat_zeros)  # compile + warm
    jax.block_until_ready(out)
    best = float("inf")
    for _ in range(timing_reps):
        t0 = time.perf_counter_ns()
        jax.block_until_ready(sharded(*concat_in, *concat_zeros))
        best = min(best, time.perf_counter_ns() - t0)
    return best


def build_current(reps=1):
    if KERNEL_V == 3:
        return _build_nc_v3(reps=reps)
    if KERNEL_V == 2:
        return _build_nc_v2(reps=reps)
    return _build_nc(MM1_MODE, reps=reps)


def benchmark(segmentations, images, reps=25, r_hi=21):
    """Estimate on-device kernel time via the replication slope: build the
    kernel with the main loop repeated 1x and r_hi times, take
    (t(r_hi) - t(1)) / (r_hi - 1). The ~100 ms axon tunnel round-trip
    cancels in the difference."""
    if KERNEL_V == 3:
        in_maps = _prep_inputs_v3(segmentations, images)
    elif KERNEL_V == 2:
        in_maps = _prep_inputs_v2(segmentations, images)
    else:
        in_maps = _prep_inputs(segmentations, images, MM1_MODE)
    times = {}
    for r in (1, r_hi):
        times[r] = _make_timer(build_current(reps=r), in_maps, reps)
    slope = (times[r_hi] - times[1]) / (r_hi - 1)
    benchmark._last = times
    return slope



# revision 23
# speedup vs baseline: 1.8244x; 1.8244x over previous
"""Trainium2 Bass kernel for DenseCRFLoss.

Computes  loss = WEIGHT * (-1/B) * sum_b  sum_{k,i,j} S[b,k,i] K_b[i,j] S[b,k,j]
where K_b[i,j] = exp(-0.5*||f_i - f_j||^2) is the joint bilateral kernel over
downsampled positions+colors (P = 96*96 = 9216 pixels per image).

Device strategy (v3, the default; v2/v1 kept for reference):
  * Features are augmented so that  fhat_i . ghat_j = -0.5*d2(i,j) in ONE
    matmul (MM1). For full precision the fp32 features are hi/lo-split into
    bf16 pairs (21-row contraction = hi.hi + hi.lo + lo.hi); the +-900
    magnitude terms cancel in PSUM fp32.
  * Symmetry: loss = 2*sum_{i<j} + sum_diag. Block-column J of an image
    gives J+1 uniform segments of 4 [128x512] tiles (w=2 above the
    diagonal, w=1 for the diagonal band); weights fold into the SJ stream.
    684 real segments + 4 dummies spread over 8 cores (NSEG=86 slots each).
  * exp is the throughput floor (one entry/cycle/lane on ACT only), so it
    is SPLIT between the scalar engine (true Exp -> bf16) and the vector
    engine: DVE groups use a Schraudolph-style fast exp -- a single
    tensor_scalar(mult,add) with fp32->uint16 saturating convert whose bit
    pattern IS the bf16 kernel value (t = x*128*log2e + 16249; negative t
    saturates to 0 = correct far-pair kernel; HW convert verified
    round-to-nearest, matching the numpy-tuned constant: +3e-4 bias at
    100% share). Share ~5/12 of groups balances ACT vs DVE busy.
  * MM2 is FLIPPED: kt tile [128,128] is the stationary (128 weight cols
    -> FWL-eligible) and S^T [128,21] is the moving operand, so MM2 costs
    only 4x21 moving columns per tile (6x less PE time than the natural
    orientation) and the As accumulator becomes [128, 4*21] in ONE PSUM
    bank. PSUM start=True zeroes the whole 2KB bank (HW-verified), so only
    the first matmul per segment starts; the other 15 accumulate.
  * Final reduce: one DVE scalar_tensor_tensor per segment (As * SJ with
    accum_out = row-sum along free dim) -> accv column; host sums.
  * 2-tile exp groups, 3 dot PSUM bufs, 4-group software pipeline so PE
    never head-of-line blocks behind exp; MM2s trail their group by 4.
  * Cost model 130.5us; HW slope 120-122us across two independent
    rotated-order measurements (v2 was 203.6us / ~176us): ~1.45x.
"""

import numpy as np
import ml_dtypes
from contextlib import ExitStack

import concourse.bass as bass
from concourse import bacc
import concourse.tile as tile
from concourse.mybir import dt, ActivationFunctionType, AluOpType, AxisListType
from concourse.bass_utils import run_bass_kernel_spmd

# ---- problem constants (hardcoded; kernel.py must be self-contained) ----
B = 4
KCH = 21
HH = 96                   # downsampled H=W
P = HH * HH               # 9216 pixels
NCORES = 8
HALF = P // 2             # 4608 columns per core
NI = P // 128             # 72 i-chunks
NJ = HALF // 512          # 9 j-chunks per core
GRP = 3                   # i-chunks per exp group (3 PSUM banks)
SIGMA_RGB = 15.0
SXY_EFF = 100.0 * 0.5     # sigma_xy * scale_factor
WEIGHT = 2e-9

MM1_MODE = "bf16split"         # "f32r" | "bf16split"

_cache = {}


def _build_nc(mm1_mode, reps=1):
    nc = bacc.Bacc("TRN2", target_bir_lowering=False)
    nf = 7 if mm1_mode == "f32r" else 21
    mm1_dt = dt.float32r if mm1_mode == "f32r" else dt.bfloat16

    fT = nc.dram_tensor("fT", [nf, P], mm1_dt, kind="ExternalInput")
    gT = nc.dram_tensor("gT", [nf, HALF], mm1_dt, kind="ExternalInput")
    sT = nc.dram_tensor("sT", [128, NI * KCH], dt.bfloat16, kind="ExternalInput")
    sj = nc.dram_tensor("sj", [KCH, HALF], dt.float32, kind="ExternalInput")
    out = nc.dram_tensor("out", [KCH, NJ], dt.float32, kind="ExternalOutput")

    with tile.TileContext(nc) as tc, ExitStack() as ctx:
        cpool = ctx.enter_context(tc.tile_pool(name="const", bufs=1))
        f_sb = cpool.tile([nf, P], mm1_dt)
        nc.gpsimd.dma_start(f_sb[:], fT[:])
        g_sb = cpool.tile([nf, HALF], mm1_dt)
        nc.gpsimd.dma_start(g_sb[:], gT[:])
        sT_sb = cpool.tile([128, NI * KCH], dt.bfloat16)
        nc.gpsimd.dma_start(sT_sb[:], sT[:])
        sj_sb = cpool.tile([KCH, HALF], dt.float32)
        nc.gpsimd.dma_start(sj_sb[:], sj[:])
        accv = cpool.tile([KCH, NJ], dt.float32)

        dpool = ctx.enter_context(tc.tile_pool(name="dot", bufs=2, space="PSUM"))
        apool = ctx.enter_context(tc.tile_pool(name="asum", bufs=2, space="PSUM"))
        kpool = ctx.enter_context(tc.tile_pool(name="ktile", bufs=3))
        spool = ctx.enter_context(tc.tile_pool(name="scr", bufs=2))

        for rep in range(reps):
            for jb in range(NJ):
                As = apool.tile([KCH, 512], dt.float32, tag="As")
                g_slice = g_sb[:, jb * 512:(jb + 1) * 512]
                for g in range(NI // GRP):
                    dot = dpool.tile([128, GRP * 512], dt.float32, tag="dot")
                    for t in range(GRP):
                        ib = g * GRP + t
                        nc.tensor.matmul(
                            dot[:, t * 512:(t + 1) * 512],
                            f_sb[:, ib * 128:(ib + 1) * 128],
                            g_slice,
                            start=True, stop=True,
                        )
                    kt = kpool.tile([128, GRP * 512], dt.bfloat16, tag="kt")
                    nc.scalar.activation(kt[:], dot[:], ActivationFunctionType.Exp)
                    for t in range(GRP):
                        ib = g * GRP + t
                        nc.tensor.matmul(
                            As[:],
                            sT_sb[:, ib * KCH:(ib + 1) * KCH],
                            kt[:, t * 512:(t + 1) * 512],
                            start=(ib == 0), stop=(ib == NI - 1),
                        )
                scr = spool.tile([KCH, 512], dt.float32, tag="scr")
                nc.vector.tensor_mul(scr[:], As[:],
                                     sj_sb[:, jb * 512:(jb + 1) * 512])
                nc.vector.reduce_sum(accv[:, jb:jb + 1], scr[:],
                                     axis=AxisListType.X)
        nc.sync.dma_start(out[:], accv[:])
    nc.finalize()
    return nc


def _split_bf16(x):
    hi = x.astype(ml_dtypes.bfloat16)
    lo = (x - hi.astype(np.float32)).astype(ml_dtypes.bfloat16)
    return hi, lo


def _prep_inputs(segmentations, images, mm1_mode):
    seg = np.asarray(segmentations, dtype=np.float32)
    img = np.asarray(images, dtype=np.float32)
    S = seg.reshape(B, KCH, HH, 2, HH, 2).mean(axis=(3, 5)).reshape(B, KCH, P)
    rgb = img[:, :, ::2, ::2].reshape(B, 3, P)

    yy, xx = np.meshgrid(np.arange(HH, dtype=np.float32),
                         np.arange(HH, dtype=np.float32), indexing="ij")
    pos = np.stack([xx.ravel(), yy.ravel()], axis=0) / SXY_EFF  # [2, P]

    in_maps = []
    for b in range(B):
        feat = np.concatenate([pos, rgb[b] / SIGMA_RGB], axis=0).astype(np.float32)
        msq = -0.5 * (feat * feat).sum(axis=0, dtype=np.float32)   # [P]
        ones = np.ones((1, P), np.float32)
        fhat = np.concatenate([feat, ones, msq[None, :]], axis=0)  # [7, P]
        ghat = np.concatenate([feat, msq[None, :], ones], axis=0)  # [7, P]
        if mm1_mode == "f32r":
            fT_full = fhat
            gT_full = ghat
        else:
            fhi, flo = _split_bf16(fhat)
            ghi, glo = _split_bf16(ghat)
            fT_full = np.concatenate([fhi, fhi, flo], axis=0)      # [21, P]
            gT_full = np.concatenate([ghi, glo, ghi], axis=0)      # [21, P]
        sT = np.ascontiguousarray(
            S[b].reshape(KCH, NI, 128).transpose(2, 1, 0).reshape(128, NI * KCH)
        ).astype(ml_dtypes.bfloat16)
        for h in range(2):
            sl = slice(h * HALF, (h + 1) * HALF)
            in_maps.append({
                "fT": np.ascontiguousarray(fT_full),
                "gT": np.ascontiguousarray(gT_full[:, sl]),
                "sT": sT,
                "sj": np.ascontiguousarray(S[b][:, sl]),
            })
    return in_maps


# ---------------- v2: symmetric (upper-triangle) kernel ----------------
# Per image, loss = 2*sum_{i<j} q_ij + sum_diag. Block-columns J (512 wide,
# 18 per image) decompose into J+1 uniform segments of 4 tiles (128 rows
# each): k<J strictly above the diagonal (weight 2), k==J the diagonal band
# (weight 1). Weights are folded into the pre-multiplied SJ stream, so the
# SPMD program is a flat, fully data-driven list of NSEG uniform segments.
NSEG = 90          # segment slots per core (684 real across 8 cores + dummies)
TPS = 4            # tiles per segment
CST = 6            # segments staged per DMA chunk
NTILE = NSEG * TPS # 360
NGRP = NTILE // GRP  # 120 exp groups of 3 tiles

def _build_nc_v2(reps=1):
    nc = bacc.Bacc("TRN2", target_bir_lowering=False)
    fD = nc.dram_tensor("fD", [21, NTILE * 128], dt.bfloat16, kind="ExternalInput")
    gD = nc.dram_tensor("gD", [21, NSEG * 512], dt.bfloat16, kind="ExternalInput")
    sjD = nc.dram_tensor("sjD", [KCH, NSEG * 512], dt.float32, kind="ExternalInput")
    stD = nc.dram_tensor("stD", [128, NTILE * KCH], dt.bfloat16, kind="ExternalInput")
    out = nc.dram_tensor("out", [KCH, NSEG], dt.float32, kind="ExternalOutput")

    with tile.TileContext(nc) as tc, ExitStack() as ctx:
        cpool = ctx.enter_context(tc.tile_pool(name="const", bufs=1))
        f_sb = cpool.tile([21, NTILE * 128], dt.bfloat16)
        nc.gpsimd.dma_start(f_sb[:], fD[:])
        st_sb = cpool.tile([128, NTILE * KCH], dt.bfloat16)
        nc.gpsimd.dma_start(st_sb[:], stD[:])
        accv = cpool.tile([KCH, NSEG], dt.float32)

        gpool = ctx.enter_context(tc.tile_pool(name="gstage", bufs=3))
        sjpool = ctx.enter_context(tc.tile_pool(name="sjstage", bufs=3))
        dpool = ctx.enter_context(tc.tile_pool(name="dot", bufs=2, space="PSUM"))
        apool = ctx.enter_context(tc.tile_pool(name="asum", bufs=2, space="PSUM"))
        kpool = ctx.enter_context(tc.tile_pool(name="ktile", bufs=3))
        spool = ctx.enter_context(tc.tile_pool(name="scr", bufs=2))

        for rep in range(reps):
            gst = {}
            sjst = {}
            As = None
            for g in range(NGRP):
                dot = dpool.tile([128, GRP * 512], dt.float32, tag="dot")
                for t in range(GRP):
                    T = g * GRP + t
                    s, pos = divmod(T, TPS)
                    c, sloc = divmod(s, CST)
                    if pos == 0 and sloc == 0:
                        gst[c] = gpool.tile([21, CST * 512], dt.bfloat16,
                                            tag="gs", name="gs")
                        nc.sync.dma_start(
                            gst[c][:],
                            gD[:, c * CST * 512:(c + 1) * CST * 512])
                        sjst[c] = sjpool.tile([KCH, CST * 512], dt.float32,
                                              tag="sjs", name="sjs")
                        nc.sync.dma_start(
                            sjst[c][:],
                            sjD[:, c * CST * 512:(c + 1) * CST * 512])
                    nc.tensor.matmul(
                        dot[:, t * 512:(t + 1) * 512],
                        f_sb[:, T * 128:(T + 1) * 128],
                        gst[c][:, sloc * 512:(sloc + 1) * 512],
                        start=True, stop=True,
                    )
                kt = kpool.tile([128, GRP * 512], dt.bfloat16, tag="kt")
                nc.scalar.activation(kt[:], dot[:], ActivationFunctionType.Exp)
                for t in range(GRP):
                    T = g * GRP + t
                    s, pos = divmod(T, TPS)
                    if pos == 0:
                        As = apool.tile([KCH, 512], dt.float32, tag="As")
                    nc.tensor.matmul(
                        As[:],
                        st_sb[:, T * KCH:(T + 1) * KCH],
                        kt[:, t * 512:(t + 1) * 512],
                        start=(pos == 0), stop=(pos == TPS - 1),
                    )
                    if pos == TPS - 1:
                        c, sloc = divmod(s, CST)
                        scr = spool.tile([KCH, 512], dt.float32, tag="scr")
                        nc.vector.tensor_mul(
                            scr[:], As[:],
                            sjst[c][:, sloc * 512:(sloc + 1) * 512])
                        nc.vector.reduce_sum(accv[:, s:s + 1], scr[:],
                                             axis=AxisListType.X)
        nc.sync.dma_start(out[:], accv[:])
    nc.finalize()
    return nc


def _prep_inputs_v2(segmentations, images):
    seg = np.asarray(segmentations, dtype=np.float32)
    img = np.asarray(images, dtype=np.float32)
    S = seg.reshape(B, KCH, HH, 2, HH, 2).mean(axis=(3, 5)).reshape(B, KCH, P)
    rgb = img[:, :, ::2, ::2].reshape(B, 3, P)
    yy, xx = np.meshgrid(np.arange(HH, dtype=np.float32),
                         np.arange(HH, dtype=np.float32), indexing="ij")
    pos = np.stack([xx.ravel(), yy.ravel()], axis=0) / SXY_EFF

    seglist = [(J, k, 2.0 if k < J else 1.0)
               for J in range(18) for k in range(J + 1)]   # 171 per image
    dummy = (0, 0, 0.0)

    in_maps = []
    for b in range(B):
        feat = np.concatenate([pos, rgb[b] / SIGMA_RGB], axis=0).astype(np.float32)
        msq = -0.5 * (feat * feat).sum(axis=0, dtype=np.float32)
        ones = np.ones((1, P), np.float32)
        fhat = np.concatenate([feat, ones, msq[None, :]], axis=0)
        ghat = np.concatenate([feat, msq[None, :], ones], axis=0)
        fhi, flo = _split_bf16(fhat)
        ghi, glo = _split_bf16(ghat)
        fT_full = np.concatenate([fhi, fhi, flo], axis=0)   # [21, P] bf16
        gT_full = np.concatenate([ghi, glo, ghi], axis=0)   # [21, P] bf16
        sT_all = np.ascontiguousarray(
            S[b].reshape(KCH, NI, 128).transpose(2, 1, 0).reshape(128, NI * KCH)
        ).astype(ml_dtypes.bfloat16)
        for h in range(2):
            segs = (seglist[:NSEG] + [dummy] * (NSEG - min(NSEG, 171))
                    if h == 0 else
                    seglist[NSEG:] + [dummy] * (2 * NSEG - 171))
            assert len(segs) == NSEG
            fDl, gDl, sjDl, stDl = [], [], [], []
            for (J, k, w) in segs:
                fDl.append(fT_full[:, 512 * k:512 * (k + 1)])
                gDl.append(gT_full[:, 512 * J:512 * (J + 1)])
                sjDl.append(np.float32(w) * S[b][:, 512 * J:512 * (J + 1)])
                stDl.append(sT_all[:, 84 * k:84 * (k + 1)])
            in_maps.append({
                "fD": np.ascontiguousarray(np.concatenate(fDl, axis=1)),
                "gD": np.ascontiguousarray(np.concatenate(gDl, axis=1)),
                "sjD": np.ascontiguousarray(
                    np.concatenate(sjDl, axis=1).astype(np.float32)),
                "stD": np.ascontiguousarray(np.concatenate(stDl, axis=1)),
            })
    return in_maps


# ---------------- v3: ACT/DVE exp split + [117,128] As layout ----------------
# Changes over v2:
#   * NSEG 90 -> 86 (684 real segments + 4 dummies over 8 cores).
#   * exp split between the scalar engine (ACT, bf16 out) and the vector
#     engine (DVE): DVE groups use a Schraudolph-style fast exp -- one
#     tensor_scalar(mult,add) with fp32->uint16 saturating convert whose
#     result bit-pattern IS the bf16 kernel value (t = x*128*log2e +
#     (127*128 + C_SCHRAUD)); negative t saturates to 0, which is the
#     correct kernel value for pairs that far apart. Constant tuned on the
#     real input distribution (rel bias +3e-4 at 100% DVE share).
#   * As accumulator [21,512] -> [117,128]: MM2 becomes 4 column-chunk
#     matmuls at tile_position column offsets 0/32/64/96, so the SJ
#     multiply-reduce runs over free-dim 128 instead of 512 and fuses into
#     ONE DVE scalar_tensor_tensor with accum_out (sum along free dim).
NSEG3 = 86
NTILE3 = NSEG3 * TPS          # 344
NGRP3 = (NTILE3 + GRP - 1) // GRP   # 115 (last group has 2 tiles)
DVE_NUM, DVE_DEN = 5, 12      # ~0.42 of exp groups routed to DVE
A_SCHRAUD = 184.66299622      # 128/ln(2)
PBASE = [0, 32, 64, 0]        # As chunk partition bases (HW allows 0/32/64)
CBASE = [0, 0, 0, 128]        # As chunk column bases
MM2_FLIP = True               # kt stationary / st moving: As' = [128j, 4*21]
B_SCHRAUD = 16249.0           # 127*128 - 7.0 (tuned on the input dist)


def _route_dve(g):
    return (g * DVE_NUM) % DVE_DEN < DVE_NUM


def _build_nc_v3(reps=1, grp=2, dot_bufs=3, kt_bufs=7, delay=4,
                 as_bufs=2, mm2_flip=True):
    grp = GRP if grp is None else grp
    nc = bacc.Bacc("TRN2", target_bir_lowering=False)
    fD = nc.dram_tensor("fD", [21, NTILE3 * 128], dt.bfloat16, kind="ExternalInput")
    gD = nc.dram_tensor("gD", [21, NSEG3 * 512], dt.bfloat16, kind="ExternalInput")
    sjw = 84 if MM2_FLIP else 256
    sjp = 128 if MM2_FLIP else 85
    sjD = nc.dram_tensor("sjD", [sjp, NSEG3 * sjw], dt.bfloat16, kind="ExternalInput")
    stD = nc.dram_tensor("stD", [128, NTILE3 * KCH], dt.bfloat16, kind="ExternalInput")
    out = nc.dram_tensor("out", [sjp, NSEG3], dt.float32, kind="ExternalOutput")

    with tile.TileContext(nc) as tc, ExitStack() as ctx:
        cpool = ctx.enter_context(tc.tile_pool(name="const", bufs=1))
        f_sb = cpool.tile([21, NTILE3 * 128], dt.bfloat16)
        for fq in range(4):   # split so early MM1s unblock before full load
            fsl = slice(fq * NTILE3 * 32, (fq + 1) * NTILE3 * 32)
            nc.gpsimd.dma_start(f_sb[:, fsl], fD[:, fsl])
        st_sb = cpool.tile([128, NTILE3 * KCH], dt.bfloat16)
        nc.gpsimd.dma_start(st_sb[:], stD[:])
        accv = cpool.tile([sjp, NSEG3], dt.float32)
        nc.vector.memset(accv[:], 0.0)

        gpool = ctx.enter_context(tc.tile_pool(name="gstage", bufs=3))
        sjpool = ctx.enter_context(tc.tile_pool(name="sjstage", bufs=3))
        dpool = ctx.enter_context(tc.tile_pool(name="dot", bufs=dot_bufs, space="PSUM"))
        apool = ctx.enter_context(tc.tile_pool(name="asum", bufs=as_bufs, space="PSUM"))
        kpool = ctx.enter_context(tc.tile_pool(name="ktile", bufs=kt_bufs))
        spool = ctx.enter_context(tc.tile_pool(name="scr", bufs=2))

        for rep in range(reps):
            gst = {}
            sjst = {}
            state = {"As": None, "kt": {}}

            def stage_mm1_exp(g):
                ntg = min(grp, NTILE3 - g * grp)
                dot = dpool.tile([128, grp * 512], dt.float32, tag="dot",
                                 name="dot")
                for t in range(ntg):
                    T = g * grp + t
                    s, pos = divmod(T, TPS)
                    c, sloc = divmod(s, CST)
                    if pos == 0 and sloc == 0:
                        nsc = min(CST, NSEG3 - c * CST)
                        gst[c] = gpool.tile([21, CST * 512], dt.bfloat16,
                                            tag="gs", name="gs")
                        nc.sync.dma_start(
                            gst[c][:, :nsc * 512],
                            gD[:, c * CST * 512:c * CST * 512 + nsc * 512])
                        sjst[c] = sjpool.tile([sjp, CST * sjw], dt.bfloat16,
                                              tag="sjs", name="sjs")
                        nc.sync.dma_start(
                            sjst[c][:, :nsc * sjw],
                            sjD[:, c * CST * sjw:c * CST * sjw + nsc * sjw])
                    nc.tensor.matmul(
                        dot[:, t * 512:(t + 1) * 512],
                        f_sb[:, T * 128:(T + 1) * 128],
                        gst[c][:, sloc * 512:(sloc + 1) * 512],
                        start=True, stop=True,
                    )
                kt = kpool.tile([128, grp * 512], dt.bfloat16, tag="kt",
                                name="kt")
                if _route_dve(g):
                    nc.vector.tensor_scalar(
                        kt[:, :ntg * 512].bitcast(dt.uint16),
                        dot[:, :ntg * 512],
                        A_SCHRAUD, B_SCHRAUD,
                        op0=AluOpType.mult, op1=AluOpType.add,
                    )
                else:
                    nc.scalar.activation(kt[:, :ntg * 512], dot[:, :ntg * 512],
                                         ActivationFunctionType.Exp)
                state["kt"][g] = kt

            def stage_mm2(g):
                ntg = min(grp, NTILE3 - g * grp)
                kt = state["kt"].pop(g)
                for t in range(ntg):
                    T = g * grp + t
                    s, pos = divmod(T, TPS)
                    if pos == 0:
                        # full-bank tile: PSUM start=True zeroes the whole
                        # 2KB bank, so (a) each buf must own its bank and
                        # (b) only the FIRST matmul per segment may start
                        state["As"] = apool.tile(
                            [128, 512], dt.float32, tag="As", name="As")
                    As = state["As"]
                    for q in range(4):
                        if MM2_FLIP:
                            # kt chunk stationary (128 weight cols -> FWL),
                            # st moving (21 cols): As'[j, 21q+k]
                            nc.tensor.matmul(
                                As[:, q * KCH:(q + 1) * KCH],
                                kt[:, t * 512 + q * 128:t * 512 + (q + 1) * 128],
                                st_sb[:, T * KCH:(T + 1) * KCH],
                                start=(pos == 0 and q == 0),
                                stop=(pos == TPS - 1),
                                skip_group_check=True,
                            )
                        else:
                            pb, cb = PBASE[q], CBASE[q]
                            nc.tensor.matmul(
                                As[pb:pb + KCH, cb:cb + 128],
                                st_sb[:, T * KCH:(T + 1) * KCH],
                                kt[:, t * 512 + q * 128:t * 512 + (q + 1) * 128],
                                start=(pos == 0), stop=(pos == TPS - 1),
                            )
                    if pos == TPS - 1:
                        c, sloc = divmod(s, CST)
                        scr = spool.tile([sjp, sjw], dt.bfloat16, tag="scr",
                                         name="scr")
                        nc.vector.scalar_tensor_tensor(
                            scr[:], As[:, :sjw], 1.0,
                            sjst[c][:, sloc * sjw:(sloc + 1) * sjw],
                            op0=AluOpType.mult, op1=AluOpType.mult,
                            accum_out=accv[:, s:s + 1],
                        )

            # software pipeline, `delay` groups deep; MM2s emitted
            # before MM1s so accums aren't queued behind the next exp
            ngrp = (NTILE3 + grp - 1) // grp
            for g in range(delay):
                stage_mm1_exp(g)
            for g in range(delay, ngrp):
                stage_mm2(g - delay)
                stage_mm1_exp(g)
            for g in range(ngrp - delay, ngrp):
                stage_mm2(g)
        nc.sync.dma_start(out[:], accv[:])
    nc.finalize()
    return nc


def _prep_inputs_v3(segmentations, images):
    seg = np.asarray(segmentations, dtype=np.float32)
    img = np.asarray(images, dtype=np.float32)
    S = seg.reshape(B, KCH, HH, 2, HH, 2).mean(axis=(3, 5)).reshape(B, KCH, P)
    rgb = img[:, :, ::2, ::2].reshape(B, 3, P)
    yy, xx = np.meshgrid(np.arange(HH, dtype=np.float32),
                         np.arange(HH, dtype=np.float32), indexing="ij")
    pos = np.stack([xx.ravel(), yy.ravel()], axis=0) / SXY_EFF

    fTs, gTs, sTs = [], [], []
    for b in range(B):
        feat = np.concatenate([pos, rgb[b] / SIGMA_RGB], axis=0).astype(np.float32)
        msq = -0.5 * (feat * feat).sum(axis=0, dtype=np.float32)
        ones = np.ones((1, P), np.float32)
        fhat = np.concatenate([feat, ones, msq[None, :]], axis=0)
        ghat = np.concatenate([feat, msq[None, :], ones], axis=0)
        fhi, flo = _split_bf16(fhat)
        ghi, glo = _split_bf16(ghat)
        fTs.append(np.concatenate([fhi, fhi, flo], axis=0))   # [21, P] bf16
        gTs.append(np.concatenate([ghi, glo, ghi], axis=0))   # [21, P] bf16
        sTs.append(np.ascontiguousarray(
            S[b].reshape(KCH, NI, 128).transpose(2, 1, 0).reshape(128, NI * KCH)
        ).astype(ml_dtypes.bfloat16))

    # global flat segment list: (b, J, k, w); 684 real + 4 dummies = 8*86
    seglist = [(b, J, k, 2.0 if k < J else 1.0)
               for b in range(B) for J in range(18) for k in range(J + 1)]
    seglist += [(0, 0, 0, 0.0)] * (NCORES * NSEG3 - len(seglist))

    in_maps = []
    for core in range(NCORES):
        segs_c = seglist[core * NSEG3:(core + 1) * NSEG3]
        fDl, gDl, sjDl, stDl = [], [], [], []
        for (b, J, k, w) in segs_c:
            fDl.append(fTs[b][:, 512 * k:512 * (k + 1)])
            gDl.append(gTs[b][:, 512 * J:512 * (J + 1)])
            if MM2_FLIP:
                # As'[j, 21q+k] -> sjb[j, 21q+k] = w*S[k, 512J+128q+j]
                blk = S[b][:, 512 * J:512 * (J + 1)]        # [21, 512]
                sjb = np.float32(w) * np.ascontiguousarray(
                    blk.reshape(KCH, 4, 128).transpose(2, 1, 0).reshape(128, 84))
            else:
                sjb = np.zeros((85, 256), np.float32)
                for q in range(4):
                    sjb[PBASE[q]:PBASE[q] + KCH, CBASE[q]:CBASE[q] + 128] = \
                        np.float32(w) * S[b][:, 512 * J + 128 * q:512 * J + 128 * (q + 1)]
            sjDl.append(sjb)
            stDl.append(sTs[b][:, 84 * k:84 * (k + 1)])
        in_maps.append({
            "fD": np.ascontiguousarray(np.concatenate(fDl, axis=1)),
            "gD": np.ascontiguousarray(np.concatenate(gDl, axis=1)),
            "sjD": np.ascontiguousarray(
                np.concatenate(sjDl, axis=1).astype(ml_dtypes.bfloat16)),
            "stD": np.ascontiguousarray(np.concatenate(stDl, axis=1)),
        })
    return in_maps


KERNEL_V = 3


def kernel(segmentations, images, _trace=False):
    if KERNEL_V == 3:
        key = "v3"
        if key not in _cache:
            _cache[key] = _build_nc_v3()
        nc = _cache[key]
        in_maps = _prep_inputs_v3(segmentations, images)
    elif KERNEL_V == 2:
        key = "v2"
        if key not in _cache:
            _cache[key] = _build_nc_v2()
        nc = _cache[key]
        in_maps = _prep_inputs_v2(segmentations, images)
    else:
        key = MM1_MODE
        if key not in _cache:
            _cache[key] = _build_nc(MM1_MODE)
        nc = _cache[key]
        in_maps = _prep_inputs(segmentations, images, MM1_MODE)
    res = run_bass_kernel_spmd(nc, in_maps, core_ids=list(range(NCORES)),
                               trace=_trace)
    kernel._last_results = res
    if KERNEL_V == 3:
        if MM2_FLIP:
            total = sum(float(np.asarray(r["out"], dtype=np.float64).sum())
                        for r in res.results)
        else:
            rows = np.r_[0:KCH, 32:32 + KCH, 64:64 + KCH]
            total = sum(float(np.asarray(r["out"], dtype=np.float64)[rows].sum())
                        for r in res.results)
    else:
        total = sum(float(np.asarray(r["out"], dtype=np.float64).sum())
                    for r in res.results)
    return np.asarray(np.float32(-WEIGHT * total / B))


def _make_runner(nc, in_maps):
    """Build the jitted SPMD executor for `nc` (mirrors
    bass2jax.run_bass_via_pjrt multi-core path) with device-resident inputs;
    return a zero-arg callable that executes once and blocks until ready."""
    import jax
    from jax.sharding import Mesh, PartitionSpec, NamedSharding
    from jax.experimental.shard_map import shard_map
    import concourse.mybir as mybir
    from concourse import bass2jax

    bass2jax.install_neuronx_cc_hook()
    partition_name = nc.partition_id_tensor.name if nc.partition_id_tensor else None
    in_names, out_names, out_avals, zero_outs = [], [], [], []
    for alloc in nc.m.functions[0].allocations:
        if not isinstance(alloc, mybir.MemoryLocationSet):
            continue
        name = alloc.memorylocations[0].name
        if alloc.kind == "ExternalInput":
            if name != partition_name:
                in_names.append(name)
        elif alloc.kind == "ExternalOutput":
            out_names.append(name)
            shape = tuple(alloc.tensor_shape)
            dtype = mybir.dt.np(alloc.dtype)
            out_avals.append(jax.core.ShapedArray(shape, dtype))
            zero_outs.append(np.zeros(shape, dtype))
    n_params = len(in_names)

    def _body(*args):
        operands = list(args)
        if partition_name is not None:
            operands.append(bass2jax.partition_id_tensor())
        outs = bass2jax._bass_exec_p.bind(
            *operands,
            out_avals=tuple(out_avals),
            in_names=tuple(in_names + out_names
                           + ([partition_name] if partition_name else [])),
            out_names=tuple(out_names),
            lowering_input_output_aliases=(),
            sim_require_finite=True,
            sim_require_nnan=True,
            nc=nc,
        )
        return tuple(outs)

    devices = jax.devices()[:NCORES]
    mesh = Mesh(np.asarray(devices), ("core",))
    in_specs = (PartitionSpec("core"),) * (n_params + len(out_names))
    out_specs = (PartitionSpec("core"),) * len(out_names)
    sharded = jax.jit(
        shard_map(_body, mesh=mesh, in_specs=in_specs, out_specs=out_specs,
                  check_rep=False),
        keep_unused=True,
    )
    per_core = [[np.asarray(m[name]) for name in in_names] for m in in_maps]
    concat_in = [
        jax.device_put(
            np.concatenate([per_core[c][i] for c in range(NCORES)], axis=0),
            NamedSharding(mesh, PartitionSpec("core")))
        for i in range(n_params)
    ]
    concat_zeros = [
        jax.device_put(np.zeros((NCORES * z.shape[0], *z.shape[1:]), z.dtype),
                       NamedSharding(mesh, PartitionSpec("core")))
        for z in zero_outs
    ]
    out = sharded(*concat_in, *concat_zeros)  # compile + warm
    jax.block_until_ready(out)

    def run():
        jax.block_until_ready(sharded(*concat_in, *concat_zeros))
    return run


def build_current(reps=1):
    if KERNEL_V == 3:
        return _build_nc_v3(reps=reps)
    if KERNEL_V == 2:
        return _build_nc_v2(reps=reps)
    return _build_nc(MM1_MODE, reps=reps)


def benchmark(segmentations, images, reps=25, r_hi=21):
    """Replication-slope estimate of on-device kernel time, robust to axon
    tunnel drift: programs with the main loop repeated r in {1, mid, r_hi}
    are timed ROUND-ROBIN (one call each per round), so slow tunnel phases
    hit all r equally within a round; the slope is the median over rounds
    of (t(r_hi) - t(1)) / (r_hi - 1)."""
    import statistics
    if KERNEL_V == 3:
        in_maps = _prep_inputs_v3(segmentations, images)
    elif KERNEL_V == 2:
        in_maps = _prep_inputs_v2(segmentations, images)
    else:
        in_maps = _prep_inputs(segmentations, images, MM1_MODE)
    r_mid = (1 + r_hi) // 2
    rs = (1, r_mid, r_hi)
    runners = {r: _make_runner(build_current(reps=r), in_maps) for r in rs}
    import time
    rounds = {r: [] for r in rs}
    for i in range(reps):
        order = rs[i % 3:] + rs[:i % 3]   # rotate to cancel order bias
        for r in order:
            t0 = time.perf_counter_ns()
            runners[r]()
            rounds[r].append(time.perf_counter_ns() - t0)
    med = {r: statistics.median(v) for r, v in rounds.items()}
    benchmark._last = {r: min(v) for r, v in rounds.items()}
    benchmark._med = med
    benchmark._slopes = {
        "hi": (med[r_hi] - med[1]) / (r_hi - 1),
        "mid": (med[r_mid] - med[1]) / (r_mid - 1),
        "top": (med[r_hi] - med[r_mid]) / (r_hi - r_mid),
    }
    return benchmark._slopes["hi"]

